# revision 19
# baseline (speedup 1.0000x reference)
"""TRN2 Bass kernel for nn_Cvx_KnapsackNet (MLP + ADMM projection QP).

Math: the reference ADMM iteration collapses to (rho=1):
    t_k     = w + |q_k|              (t_0 = w)
    x_k     = t_k @ P' + c           (P' = (I - A^T(AA^T)^-1 A)/2, bias-row trick for c)
    q_{k+1} = q_k + alpha*(x_k - max(q_k, 0))   (over-relaxed, alpha=1.8)
Over-relaxation (alpha=1.8) more than doubles the contraction rate vs the
reference's alpha=1, so 5 bf16 iterations reach rel err ~6.3e-3 vs the
200-iter reference (gate 2e-2); floor is the bf16 MLP (~1.6e-3). Iteration 0
exploits t_0 = w being zero outside tiles 0..3 and the bias row.

All heavy operands are bf16: W2 (20.5 MB streamed in 10 chunked DMAs,
pipelined four-deep on the sync queue while small inputs ride the scalar
queue), W3, P (j-major, 3 tiles so ADMM groups start as slices land).
ADMM x lands in 3-slot PSUM bank tiles ([128,384]) and the elementwise
update runs as 384-wide grouped bf16 ops across vector/scalar/gpsimd.

Sharding: pure data parallel, batch 1024 -> 128 rows per NeuronCore.
On-chip layout is transposed ([n2p=1152 rows, 128 batch cols], 9 tiles
of 128 partitions) so the matmul contraction runs over partitions.
"""
import sys
sys.path.insert(0, '/opt/trn_rl_repo')
import os
import numpy as np

B, C, H, R, K = 1024, 32, 3200, 500, 30
RHO = 1.0
N1 = K + R              # 530
N2 = R + K + R          # 1030
N2P = 1152              # 9 * 128
NT = N2P // 128         # 9 state tiles
BIAS_ROW = N2           # 1030
NCORES = 8
BL = B // NCORES        # 128 batch rows per core
HT = H // 128           # 25 hidden tiles
ITERS = int(os.environ.get("KNAP_ITERS", "5"))
ALPHA = float(os.environ.get("KNAP_ALPHA", "1.8"))
MC_W = 5                # m-tiles per W2 chunk
N_MC = HT // MC_W       # 5 chunks
CHUNK = HT * MC_W * 128  # 16000 cols per W2 mc-block
KSPLIT = 13             # k-tiles 0..12 in first half-chunk, 13..24 in second
CT = 512 // 128         # 4 cost tiles (500 padded to 512)
PG = 3 * NT * 128       # P columns per j-group tile (3456)

_CACHE = {}


def _host_precompute(W1, b1, W2, b2, W3, b3, weights_mat, capacities):
    """float64 host math -> packed fp32/bf16 device constants."""
    import ml_dtypes
    bf16 = ml_dtypes.bfloat16
    wm = weights_mat.astype(np.float64)
    cap = capacities.astype(np.float64)
    A = np.zeros((N1, N2), np.float64)
    A[:K, :R] = wm
    A[:K, R:R + K] = np.eye(K)
    A[K:, :R] = np.eye(R)
    A[K:, R + K:] = np.eye(R)
    b = np.concatenate([cap, np.ones(R)])
    M = np.linalg.inv(A @ A.T)
    P = (np.eye(N2) - A.T @ M @ A) / (1.0 + RHO)
    c = b @ M @ A
    Pbig = np.zeros((N2P, N2P), np.float32)
    Pbig[:N2, :N2] = P.astype(np.float32)
    Pbig[BIAS_ROW, :N2] = c.astype(np.float32)
    # j-major blocked: PbigPM[p, (j*NT+k)*128 + f] = Pbig[k*128+p, j*128+f]
    PbigPM = np.ascontiguousarray(
        Pbig.reshape(NT, 128, NT, 128).transpose(1, 2, 0, 3).reshape(128, NT * NT * 128))
    PbigBF = PbigPM.astype(bf16)

    W3p = np.zeros((512, H), np.float32)
    W3p[:R] = W3
    # w3PM[p, h*512 + f] = W3p.T[h*128+p, f]
    w3PM = np.ascontiguousarray(
        W3p.T.reshape(HT, 128, 512).transpose(1, 0, 2).reshape(128, HT * 512)).astype(bf16)

    b1R = np.ascontiguousarray(b1.reshape(HT, 128).T)       # [128, 25]
    b2R = np.ascontiguousarray(b2.reshape(HT, 128).T)       # [128, 25]
    b3p = np.zeros(512, np.float32)
    b3p[:R] = b3
    b3R = np.ascontiguousarray(b3p.reshape(CT, 128).T)      # [128, 4]
    # padding tiles 4..8 of w (zeros; bias-row 1030 -> tile 8, partition 6 = 1)
    wpad = np.zeros((128, (NT - CT) * 128), np.float32)
    wpad[BIAS_ROW - 8 * 128, (8 - CT) * 128:(9 - CT) * 128] = 1.0

    small = np.concatenate([b1R, b2R, b3R, wpad], axis=1).astype(np.float32)
    W1T = np.ascontiguousarray(W1.T)                        # [32, 3200]
    # W2 blocked so each (mc, k-range) chunk is one contiguous DMA:
    # W2PK[p, mc*CHUNK + k*640 + f] = W2T[k*128+p, mc*640+f]
    W2T = np.ascontiguousarray(W2.T)                        # [3200, 3200]
    W2PK = np.ascontiguousarray(
        W2T.reshape(HT, 128, N_MC, MC_W * 128).transpose(1, 2, 0, 3)
           .reshape(128, H * HT)).astype(bf16)
    return small, PbigBF, w3PM, W1T, W2PK


def _build_nc():
    import concourse.bacc as bacc
    import concourse.mybir as mybir
    from concourse import tile
    from concourse.tile_rust import add_dep_helper

    f32 = mybir.dt.float32
    bf16 = mybir.dt.bfloat16
    SMALL_W = HT + HT + CT + (NT - CT) * 128
    OFF_B1 = 0
    OFF_B2 = OFF_B1 + HT
    OFF_B3 = OFF_B2 + HT
    OFF_WP = OFF_B3 + CT
    CH_A = KSPLIT * MC_W * 128            # 8320 cols (k 0..12)
    CH_B = (HT - KSPLIT) * MC_W * 128     # 7680 cols (k 13..24)

    nc = bacc.Bacc("TRN2", target_bir_lowering=False, debug=False, num_devices=NCORES)
    small_d = nc.dram_tensor("small_d", [128, SMALL_W], f32, kind="ExternalInput").ap()
    pbf_d = nc.dram_tensor("pbf_d", [128, NT * NT * 128], bf16, kind="ExternalInput").ap()
    w3_d = nc.dram_tensor("w3_d", [128, HT * 512], bf16, kind="ExternalInput").ap()
    dw_d = nc.dram_tensor("dw_d", [C, BL + H], f32, kind="ExternalInput").ap()
    w2_d = nc.dram_tensor("w2_d", [128, H * HT], bf16, kind="ExternalInput").ap()
    out_d = nc.dram_tensor("out_d", [128, N2P], f32, kind="ExternalOutput").ap()

    Act = mybir.ActivationFunctionType
    Alu = mybir.AluOpType

    with tile.TileContext(nc) as tc:
        with tc.tile_pool(name="sb", bufs=1) as sb, \
             tc.tile_pool(name="wst", bufs=5) as wst, \
             tc.tile_pool(name="mlp", bufs=1) as mlp, \
             tc.tile_pool(name="ps", bufs=8, space="PSUM") as pspool:
            # W2 chunk 0 leads the idle gpsimd queue (sync's preamble delays
            # its first DMA ~6us); small inputs ride the scalar queue
            w2blk0 = wst.tile([128, CH_A], bf16, name="w2blk")
            nc.gpsimd.dma_start(out=w2blk0[:, 0:CH_A], in_=w2_d[:, 0:CH_A])
            dw = mlp.tile([C, BL + H], f32)
            nc.scalar.dma_start(out=dw[:], in_=dw_d[:])
            sm = sb.tile([128, SMALL_W], f32)
            nc.scalar.dma_start(out=sm[:], in_=small_d[:])
            pbfg = [sb.tile([128, PG], bf16, name=f"pbf{g}") for g in range(3)]
            w3t = sb.tile([128, HT * 512], bf16)

            b1R = sm[:, OFF_B1:OFF_B1 + HT]
            b2R = sm[:, OFF_B2:OFF_B2 + HT]
            b3R = sm[:, OFF_B3:OFF_B3 + CT]
            dT = dw[:, 0:BL]
            w1T = dw[:, BL:BL + H]

            h1 = mlp.tile([128, HT * 128], bf16)  # h1T tiles: [p, m*128+b]
            h2 = mlp.tile([128, HT * 128], bf16)
            w_sb = sb.tile([128, N2P], f32)       # wT tiles: [p, j*128+b]
            wb_sb = sb.tile([128, N2P], bf16)
            q_sb = sb.tile([128, N2P], bf16)
            a_sb = sb.tile([128, N2P], bf16)
            r_sb = sb.tile([128, N2P], bf16)
            t_bufs = [sb.tile([128, N2P], bf16, name=f"t{i}") for i in range(3)]
            out_g = [sb.tile([128, 384], f32, name=f"og{g}") for g in range(3)]

            nc.vector.tensor_copy(w_sb[:, CT * 128:],
                                  sm[:, OFF_WP:OFF_WP + (NT - CT) * 128])

            # ---- MLP layer 1 (fp32): h1T[m] = prelu(W1T[:,m].T @ dT + b1, 0.1) ----
            for m in range(HT):
                ps_t = pspool.tile([128, 128], f32, tag="ps", name="ps_t")
                nc.tensor.matmul(ps_t[:], w1T[:, m * 128:(m + 1) * 128], dT,
                                 start=True, stop=True)
                nc.scalar.activation(h1[:, m * 128:(m + 1) * 128], ps_t[:],
                                     Act.Prelu, bias=b1R[:, m:m + 1], alpha=0.1)

            # ---- MLP layer 2 (bf16): 10 chunked DMAs, pipelined four-deep ----
            chunk_marks = []   # first matmul touching each chunk
            ci = 0
            for mc in range(N_MC):
                ps_list = [pspool.tile([128, 128], f32, tag="ps", name="ps_t")
                           for _ in range(MC_W)]
                for half, (k0, k1, coff, cw) in enumerate(
                        [(0, KSPLIT, 0, CH_A), (KSPLIT, HT, CH_A, CH_B)]):
                    if ci == 0:
                        w2blk = w2blk0
                    else:
                        w2blk = wst.tile([128, CH_A], bf16, name="w2blk")
                        w2dma = nc.sync.dma_start(
                            out=w2blk[:, 0:cw],
                            in_=w2_d[:, mc * CHUNK + coff:mc * CHUNK + coff + cw])
                        if ci >= 4:
                            add_dep_helper(w2dma.ins, chunk_marks[ci - 4], sync=True,
                                           reason="four-behind W2 chunk pipeline")
                    for k in range(k0, k1):
                        kb = (k - k0) * MC_W * 128
                        for mi in range(MC_W):
                            mm = nc.tensor.matmul(ps_list[mi][:],
                                             w2blk[:, kb + mi * 128:kb + (mi + 1) * 128],
                                             h1[:, k * 128:(k + 1) * 128],
                                             start=(k == 0), stop=(k == HT - 1))
                            if k == k0 and mi == 0:
                                chunk_marks.append(mm.ins)
                    ci += 1
                for mi in range(MC_W):
                    m = mc * MC_W + mi
                    nc.scalar.activation(h2[:, m * 128:(m + 1) * 128], ps_list[mi][:],
                                         Act.Prelu, bias=b2R[:, m:m + 1], alpha=0.1)

            # W3 then P (3 j-group slices) stream in behind the W2 tail
            w3dma = nc.sync.dma_start(out=w3t[:], in_=w3_d[:])
            add_dep_helper(w3dma.ins, chunk_marks[8], sync=True,
                           reason="W3 load behind tail of W2 stream")
            for g in range(3):
                pdma = nc.sync.dma_start(out=pbfg[g][:],
                                         in_=pbf_d[:, g * PG:(g + 1) * PG])
                add_dep_helper(pdma.ins, chunk_marks[9], sync=True,
                               reason="P load after last W2 chunk")

            # ---- cost layer (bf16): w tiles 0..3 = sum_h W3p.T[h] @ h2T[h] + b3 ----
            ps_cost = [pspool.tile([128, 128], f32, tag="ps", name="ps_t")
                       for _ in range(CT)]
            for h in range(HT):
                for m in range(CT):
                    nc.tensor.matmul(ps_cost[m][:],
                                     w3t[:, h * 512 + m * 128:h * 512 + (m + 1) * 128],
                                     h2[:, h * 128:(h + 1) * 128],
                                     start=(h == 0), stop=(h == HT - 1))
            for m in range(CT):
                nc.scalar.activation(w_sb[:, m * 128:(m + 1) * 128], ps_cost[m][:],
                                     Act.Identity, bias=b3R[:, m:m + 1])
            nc.vector.tensor_copy(wb_sb[:], w_sb[:])

            # ---- ADMM iterations (bf16 matmuls + bf16 grouped elementwise) ----
            # t_0 = w is zero outside tiles 0..3 (cost) and 8 (bias row), so
            # iteration 0 contracts only those 5 k-tiles and sets q = alpha*x.
            for it in range(ITERS):
                last = (it == ITERS - 1)
                cur = wb_sb if it == 0 else t_bufs[(it - 1) % 3]
                klist = [0, 1, 2, 3, 8] if it == 0 else list(range(NT))
                for g in range(3):
                    ps_g = pspool.tile([128, 384], f32, tag="ps", name="ps_g")
                    for js in range(3):
                        j = g * 3 + js
                        for ki, k in enumerate(klist):
                            nc.tensor.matmul(ps_g[:, js * 128:(js + 1) * 128],
                                             pbfg[g][:, (js * NT + k) * 128:(js * NT + k + 1) * 128],
                                             cur[:, k * 128:(k + 1) * 128],
                                             start=(ki == 0), stop=(ki == len(klist) - 1))
                    gg = slice(g * 384, (g + 1) * 384)
                    if last:
                        nc.scalar.activation(out_g[g][:], ps_g[:], Act.Copy)
                        nc.sync.dma_start(out=out_d[:, gg], in_=out_g[g][:])
                    else:
                        if it == 0:
                            # q = alpha*x  (q starts at 0)
                            nc.vector.tensor_scalar_mul(
                                out=q_sb[:, gg], in0=ps_g[:], scalar1=ALPHA)
                        else:
                            # r = max(q,0) - x;  q += -alpha*r
                            nc.vector.scalar_tensor_tensor(
                                out=r_sb[:, gg], in0=q_sb[:, gg], scalar=0.0,
                                in1=ps_g[:], op0=Alu.max, op1=Alu.subtract)
                            nc.vector.scalar_tensor_tensor(
                                out=q_sb[:, gg], in0=r_sb[:, gg], scalar=-ALPHA,
                                in1=q_sb[:, gg], op0=Alu.mult, op1=Alu.add)
                        # a = |q|;  t = a + w
                        if g == 2:
                            # last group: keep the whole tail on vector
                            # (|q| = max(-q, q)) to skip cross-engine hops
                            nc.vector.scalar_tensor_tensor(
                                out=a_sb[:, gg], in0=q_sb[:, gg], scalar=-1.0,
                                in1=q_sb[:, gg], op0=Alu.mult, op1=Alu.max)
                            nc.vector.tensor_tensor(out=t_bufs[it % 3][:, gg],
                                                    in0=a_sb[:, gg], in1=wb_sb[:, gg],
                                                    op=Alu.add)
                        else:
                            nc.scalar.activation(a_sb[:, gg], q_sb[:, gg], Act.Abs)
                            if g == 0:
                                nc.gpsimd.tensor_tensor(out=t_bufs[it % 3][:, gg],
                                                        in0=a_sb[:, gg], in1=wb_sb[:, gg],
                                                        op=Alu.add)
                            else:
                                nc.vector.tensor_tensor(out=t_bufs[it % 3][:, gg],
                                                        in0=a_sb[:, gg], in1=wb_sb[:, gg],
                                                        op=Alu.add)

    nc.compile()
    return nc


def kernel(d, W1, b1, W2, b2, W3, b3, weights_mat, capacities):
    from concourse.bass_utils import run_bass_kernel_spmd

    d = np.asarray(d, np.float32)
    small, PbigBF, w3PM, W1T, W2PK = _host_precompute(
        np.asarray(W1, np.float32), np.asarray(b1, np.float32),
        np.asarray(W2, np.float32), np.asarray(b2, np.float32),
        np.asarray(W3, np.float32), np.asarray(b3, np.float32),
        np.asarray(weights_mat, np.float32), np.asarray(capacities, np.float32))

    if "nc" not in _CACHE:
        _CACHE["nc"] = _build_nc()
    nc = _CACHE["nc"]

    in_maps = []
    for i in range(NCORES):
        dTc = np.ascontiguousarray(d[i * BL:(i + 1) * BL].T)      # [32, 128]
        dwc = np.concatenate([dTc, W1T], axis=1)                  # [32, 128+3200]
        in_maps.append({"small_d": small, "pbf_d": PbigBF,
                        "w3_d": w3PM, "dw_d": dwc, "w2_d": W2PK})

    trace = bool(int(os.environ.get("KNAP_TRACE", "0")))
    res = run_bass_kernel_spmd(nc, in_maps, core_ids=list(range(NCORES)),
                               trace=trace)
    if trace:
        _CACHE["exec_time_ns"] = res.exec_time_ns
        _CACHE["trace"] = res.instructions_and_trace

    out = np.empty((B, N2), np.float32)
    for i in range(NCORES):
        arr = res.results[i]["out_d"]                              # [128, 1152]
        xc = arr.reshape(128, NT, 128).transpose(2, 1, 0).reshape(BL, N2P)
        out[i * BL:(i + 1) * BL] = xc[:, :N2]
    return out


# revision 21
# speedup vs baseline: 1.1077x; 1.1077x over previous
"""TRN2 Bass kernel for nn_Cvx_KnapsackNet (MLP + ADMM projection QP).

Math: the reference ADMM iteration collapses to (rho=1):
    t_k     = w + |q_k|              (t_0 = w)
    x_k     = t_k @ P' + c           (P' = (I - A^T(AA^T)^-1 A)/2, bias-row trick for c)
    q_{k+1} = q_k + alpha*(x_k - max(q_k, 0))   (over-relaxed, alpha=1.8)
Over-relaxation (alpha=1.8) more than doubles the contraction rate vs the
reference's alpha=1, so 5 bf16 iterations reach rel err ~6.3e-3 vs the
200-iter reference (gate 2e-2); floor is the bf16 MLP (~1.6e-3). Iteration 0
exploits t_0 = w being zero outside tiles 0..3 and the bias row.

All heavy operands are bf16: W2 (20.5 MB streamed in 10 chunked DMAs,
pipelined four-deep on the sync queue while small inputs ride the scalar
queue), W3, P (j-major, 3 tiles so ADMM groups start as slices land).
ADMM x lands in 3-slot PSUM bank tiles ([128,384]) and the elementwise
update runs as 384-wide grouped bf16 ops across vector/scalar/gpsimd.

Sharding: pure data parallel, batch 1024 -> 128 rows per NeuronCore.
On-chip layout is transposed ([n2p=1152 rows, 128 batch cols], 9 tiles
of 128 partitions) so the matmul contraction runs over partitions.
"""
import sys
sys.path.insert(0, '/opt/trn_rl_repo')
import os
import numpy as np

B, C, H, R, K = 1024, 32, 3200, 500, 30
RHO = 1.0
N1 = K + R              # 530
N2 = R + K + R          # 1030
N2P = 1152              # 9 * 128
NT = N2P // 128         # 9 state tiles
BIAS_ROW = N2           # 1030
NCORES = 8
BL = B // NCORES        # 128 batch rows per core
HT = H // 128           # 25 hidden tiles
ITERS = int(os.environ.get("KNAP_ITERS", "5"))
ALPHA = float(os.environ.get("KNAP_ALPHA", "1.8"))
MC_W = 5                # m-tiles per W2 chunk
N_MC = HT // MC_W       # 5 chunks
CHUNK = HT * MC_W * 128  # 16000 cols per W2 mc-block
KSPLIT = 13             # k-tiles 0..12 in first half-chunk, 13..24 in second
CT = 512 // 128         # 4 cost tiles (500 padded to 512)
PG = 3 * NT * 128       # P columns per j-group tile (3456)

_CACHE = {}


def _host_precompute(W1, b1, W2, b2, W3, b3, weights_mat, capacities):
    """float64 host math -> packed fp32/bf16 device constants."""
    import ml_dtypes
    bf16 = ml_dtypes.bfloat16
    wm = weights_mat.astype(np.float64)
    cap = capacities.astype(np.float64)
    A = np.zeros((N1, N2), np.float64)
    A[:K, :R] = wm
    A[:K, R:R + K] = np.eye(K)
    A[K:, :R] = np.eye(R)
    A[K:, R + K:] = np.eye(R)
    b = np.concatenate([cap, np.ones(R)])
    M = np.linalg.inv(A @ A.T)
    P = (np.eye(N2) - A.T @ M @ A) / (1.0 + RHO)
    c = b @ M @ A
    Pbig = np.zeros((N2P, N2P), np.float32)
    Pbig[:N2, :N2] = P.astype(np.float32)
    Pbig[BIAS_ROW, :N2] = c.astype(np.float32)
    # j-major blocked: PbigPM[p, (j*NT+k)*128 + f] = Pbig[k*128+p, j*128+f]
    PbigPM = np.ascontiguousarray(
        Pbig.reshape(NT, 128, NT, 128).transpose(1, 2, 0, 3).reshape(128, NT * NT * 128))
    PbigBF = PbigPM.astype(bf16)

    W3p = np.zeros((512, H), np.float32)
    W3p[:R] = W3
    # w3PM[p, h*512 + f] = W3p.T[h*128+p, f]
    w3PM = np.ascontiguousarray(
        W3p.T.reshape(HT, 128, 512).transpose(1, 0, 2).reshape(128, HT * 512)).astype(bf16)

    b1R = np.ascontiguousarray(b1.reshape(HT, 128).T)       # [128, 25]
    b2R = np.ascontiguousarray(b2.reshape(HT, 128).T)       # [128, 25]
    b3p = np.zeros(512, np.float32)
    b3p[:R] = b3
    b3R = np.ascontiguousarray(b3p.reshape(CT, 128).T)      # [128, 4]
    # padding tiles 4..8 of w (zeros; bias-row 1030 -> tile 8, partition 6 = 1)
    wpad = np.zeros((128, (NT - CT) * 128), np.float32)
    wpad[BIAS_ROW - 8 * 128, (8 - CT) * 128:(9 - CT) * 128] = 1.0

    small = np.concatenate([b1R, b2R, b3R, wpad], axis=1).astype(np.float32)
    W1T = np.ascontiguousarray(W1.T)                        # [32, 3200]
    # W2 blocked so each (mc, k-range) chunk is one contiguous DMA:
    # W2PK[p, mc*CHUNK + k*640 + f] = W2T[k*128+p, mc*640+f]
    W2T = np.ascontiguousarray(W2.T)                        # [3200, 3200]
    W2PK = np.ascontiguousarray(
        W2T.reshape(HT, 128, N_MC, MC_W * 128).transpose(1, 2, 0, 3)
           .reshape(128, H * HT)).astype(bf16)
    return small, PbigBF, w3PM, W1T, W2PK


def _build_nc():
    import concourse.bacc as bacc
    import concourse.mybir as mybir
    from concourse import tile
    from concourse.tile_rust import add_dep_helper

    f32 = mybir.dt.float32
    bf16 = mybir.dt.bfloat16
    SMALL_W = HT + HT + CT + (NT - CT) * 128
    OFF_B1 = 0
    OFF_B2 = OFF_B1 + HT
    OFF_B3 = OFF_B2 + HT
    OFF_WP = OFF_B3 + CT
    CH_A = KSPLIT * MC_W * 128            # 8320 cols (k 0..12)
    CH_B = (HT - KSPLIT) * MC_W * 128     # 7680 cols (k 13..24)

    nc = bacc.Bacc("TRN2", target_bir_lowering=False, debug=False, num_devices=NCORES)
    small_d = nc.dram_tensor("small_d", [128, SMALL_W], f32, kind="ExternalInput").ap()
    pbf_d = nc.dram_tensor("pbf_d", [128, NT * NT * 128], bf16, kind="ExternalInput").ap()
    w3_d = nc.dram_tensor("w3_d", [128, HT * 512], bf16, kind="ExternalInput").ap()
    dw_d = nc.dram_tensor("dw_d", [C, BL + H], f32, kind="ExternalInput").ap()
    w2_d = nc.dram_tensor("w2_d", [128, H * HT], bf16, kind="ExternalInput").ap()
    out_d = nc.dram_tensor("out_d", [128, N2P], f32, kind="ExternalOutput").ap()

    Act = mybir.ActivationFunctionType
    Alu = mybir.AluOpType

    with tile.TileContext(nc) as tc:
        with tc.tile_pool(name="sb", bufs=1) as sb, \
             tc.tile_pool(name="wst", bufs=6) as wst, \
             tc.tile_pool(name="mlp", bufs=1) as mlp, \
             tc.tile_pool(name="ps", bufs=8, space="PSUM") as pspool:
            # W2 chunk 0 leads the sync queue; small inputs ride the scalar queue
            w2blk0 = wst.tile([128, CH_A], bf16, name="w2blk")
            nc.sync.dma_start(out=w2blk0[:, 0:CH_A], in_=w2_d[:, 0:CH_A])
            dw = mlp.tile([C, BL + H], f32)
            nc.scalar.dma_start(out=dw[:], in_=dw_d[:])
            sm = sb.tile([128, SMALL_W], f32)
            nc.scalar.dma_start(out=sm[:], in_=small_d[:])
            pbfg = [sb.tile([128, PG], bf16, name=f"pbf{g}") for g in range(3)]
            w3t = sb.tile([128, HT * 512], bf16)

            b1R = sm[:, OFF_B1:OFF_B1 + HT]
            b2R = sm[:, OFF_B2:OFF_B2 + HT]
            b3R = sm[:, OFF_B3:OFF_B3 + CT]
            dT = dw[:, 0:BL]
            w1T = dw[:, BL:BL + H]

            h1 = mlp.tile([128, HT * 128], bf16)  # h1T tiles: [p, m*128+b]
            h2 = mlp.tile([128, HT * 128], bf16)
            w_sb = sb.tile([128, N2P], f32)       # wT tiles: [p, j*128+b]
            wb_sb = sb.tile([128, N2P], bf16)
            q_sb = sb.tile([128, N2P], bf16)
            a_sb = sb.tile([128, N2P], bf16)
            r_sb = sb.tile([128, N2P], bf16)
            t_bufs = [sb.tile([128, N2P], bf16, name=f"t{i}") for i in range(3)]
            out_g = [sb.tile([128, 384], f32, name=f"og{g}") for g in range(3)]

            nc.vector.tensor_copy(w_sb[:, CT * 128:],
                                  sm[:, OFF_WP:OFF_WP + (NT - CT) * 128])

            # ---- MLP layer 1 (fp32): h1T[m] = prelu(W1T[:,m].T @ dT + b1, 0.1) ----
            for m in range(HT):
                ps_t = pspool.tile([128, 128], f32, tag="ps", name="ps_t")
                nc.tensor.matmul(ps_t[:], w1T[:, m * 128:(m + 1) * 128], dT,
                                 start=True, stop=True)
                nc.scalar.activation(h1[:, m * 128:(m + 1) * 128], ps_t[:],
                                     Act.Prelu, bias=b1R[:, m:m + 1], alpha=0.1)

            # ---- MLP layer 2 (bf16): 10 chunked DMAs, pipelined four-deep ----
            chunk_marks = []   # first matmul touching each chunk
            ci = 0
            for mc in range(N_MC):
                ps_list = [pspool.tile([128, 128], f32, tag="ps", name="ps_t")
                           for _ in range(MC_W)]
                for half, (k0, k1, coff, cw) in enumerate(
                        [(0, KSPLIT, 0, CH_A), (KSPLIT, HT, CH_A, CH_B)]):
                    if ci == 0:
                        w2blk = w2blk0
                    else:
                        w2blk = wst.tile([128, CH_A], bf16, name="w2blk")
                        w2dma = nc.sync.dma_start(
                            out=w2blk[:, 0:cw],
                            in_=w2_d[:, mc * CHUNK + coff:mc * CHUNK + coff + cw])
                        if ci >= 5:
                            add_dep_helper(w2dma.ins, chunk_marks[ci - 5], sync=True,
                                           reason="five-behind W2 chunk pipeline")
                    for k in range(k0, k1):
                        kb = (k - k0) * MC_W * 128
                        for mi in range(MC_W):
                            mm = nc.tensor.matmul(ps_list[mi][:],
                                             w2blk[:, kb + mi * 128:kb + (mi + 1) * 128],
                                             h1[:, k * 128:(k + 1) * 128],
                                             start=(k == 0), stop=(k == HT - 1))
                            if k == k0 and mi == 0:
                                chunk_marks.append(mm.ins)
                    ci += 1
                for mi in range(MC_W):
                    m = mc * MC_W + mi
                    nc.scalar.activation(h2[:, m * 128:(m + 1) * 128], ps_list[mi][:],
                                         Act.Prelu, bias=b2R[:, m:m + 1], alpha=0.1)

            # W3 then P (3 j-group slices) stream in behind the W2 tail
            w3dma = nc.sync.dma_start(out=w3t[:], in_=w3_d[:])
            add_dep_helper(w3dma.ins, chunk_marks[8], sync=True,
                           reason="W3 load behind tail of W2 stream")
            for g in range(3):
                pdma = nc.sync.dma_start(out=pbfg[g][:],
                                         in_=pbf_d[:, g * PG:(g + 1) * PG])
                add_dep_helper(pdma.ins, chunk_marks[9], sync=True,
                               reason="P load after last W2 chunk")

            # ---- cost layer (bf16): w tiles 0..3 = sum_h W3p.T[h] @ h2T[h] + b3 ----
            ps_cost = [pspool.tile([128, 128], f32, tag="ps", name="ps_t")
                       for _ in range(CT)]
            for h in range(HT):
                for m in range(CT):
                    nc.tensor.matmul(ps_cost[m][:],
                                     w3t[:, h * 512 + m * 128:h * 512 + (m + 1) * 128],
                                     h2[:, h * 128:(h + 1) * 128],
                                     start=(h == 0), stop=(h == HT - 1))
            for m in range(CT):
                nc.scalar.activation(w_sb[:, m * 128:(m + 1) * 128], ps_cost[m][:],
                                     Act.Identity, bias=b3R[:, m:m + 1])
            nc.vector.tensor_copy(wb_sb[:], w_sb[:])

            # ---- ADMM iterations (bf16 matmuls + bf16 grouped elementwise) ----
            # t_0 = w is zero outside tiles 0..3 (cost) and 8 (bias row), so
            # iteration 0 contracts only those 5 k-tiles and sets q = alpha*x.
            for it in range(ITERS):
                last = (it == ITERS - 1)
                cur = wb_sb if it == 0 else t_bufs[(it - 1) % 3]
                klist = [0, 1, 2, 3, 8] if it == 0 else list(range(NT))
                for g in range(3):
                    ps_g = pspool.tile([128, 384], f32, tag="ps", name="ps_g")
                    for js in range(3):
                        j = g * 3 + js
                        for ki, k in enumerate(klist):
                            nc.tensor.matmul(ps_g[:, js * 128:(js + 1) * 128],
                                             pbfg[g][:, (js * NT + k) * 128:(js * NT + k + 1) * 128],
                                             cur[:, k * 128:(k + 1) * 128],
                                             start=(ki == 0), stop=(ki == len(klist) - 1))
                    gg = slice(g * 384, (g + 1) * 384)
                    if last:
                        nc.scalar.activation(out_g[g][:], ps_g[:], Act.Copy)
                        nc.sync.dma_start(out=out_d[:, gg], in_=out_g[g][:])
                    else:
                        if it == 0:
                            # q = alpha*x  (q starts at 0)
                            nc.vector.tensor_scalar_mul(
                                out=q_sb[:, gg], in0=ps_g[:], scalar1=ALPHA)
                        else:
                            # r = max(q,0) - x;  q += -alpha*r
                            nc.vector.scalar_tensor_tensor(
                                out=r_sb[:, gg], in0=q_sb[:, gg], scalar=0.0,
                                in1=ps_g[:], op0=Alu.max, op1=Alu.subtract)
                            nc.vector.scalar_tensor_tensor(
                                out=q_sb[:, gg], in0=r_sb[:, gg], scalar=-ALPHA,
                                in1=q_sb[:, gg], op0=Alu.mult, op1=Alu.add)
                        # a = |q|;  t = a + w
                        if g == 2:
                            # last group: keep the whole tail on vector
                            # (|q| = max(-q, q)) to skip cross-engine hops
                            nc.vector.scalar_tensor_tensor(
                                out=a_sb[:, gg], in0=q_sb[:, gg], scalar=-1.0,
                                in1=q_sb[:, gg], op0=Alu.mult, op1=Alu.max)
                            nc.vector.tensor_tensor(out=t_bufs[it % 3][:, gg],
                                                    in0=a_sb[:, gg], in1=wb_sb[:, gg],
                                                    op=Alu.add)
                        else:
                            nc.scalar.activation(a_sb[:, gg], q_sb[:, gg], Act.Abs)
                            if g == 0:
                                nc.gpsimd.tensor_tensor(out=t_bufs[it % 3][:, gg],
                                                        in0=a_sb[:, gg], in1=wb_sb[:, gg],
                                                        op=Alu.add)
                            else:
                                nc.vector.tensor_tensor(out=t_bufs[it % 3][:, gg],
                                                        in0=a_sb[:, gg], in1=wb_sb[:, gg],
                                                        op=Alu.add)

    nc.compile()
    return nc


def kernel(d, W1, b1, W2, b2, W3, b3, weights_mat, capacities):
    from concourse.bass_utils import run_bass_kernel_spmd

    d = np.asarray(d, np.float32)
    small, PbigBF, w3PM, W1T, W2PK = _host_precompute(
        np.asarray(W1, np.float32), np.asarray(b1, np.float32),
        np.asarray(W2, np.float32), np.asarray(b2, np.float32),
        np.asarray(W3, np.float32), np.asarray(b3, np.float32),
        np.asarray(weights_mat, np.float32), np.asarray(capacities, np.float32))

    if "nc" not in _CACHE:
        _CACHE["nc"] = _build_nc()
    nc = _CACHE["nc"]

    in_maps = []
    for i in range(NCORES):
        dTc = np.ascontiguousarray(d[i * BL:(i + 1) * BL].T)      # [32, 128]
        dwc = np.concatenate([dTc, W1T], axis=1)                  # [32, 128+3200]
        in_maps.append({"small_d": small, "pbf_d": PbigBF,
                        "w3_d": w3PM, "dw_d": dwc, "w2_d": W2PK})

    trace = bool(int(os.environ.get("KNAP_TRACE", "0")))
    res = run_bass_kernel_spmd(nc, in_maps, core_ids=list(range(NCORES)),
                               trace=trace)
    if trace:
        _CACHE["exec_time_ns"] = res.exec_time_ns
        _CACHE["trace"] = res.instructions_and_trace

    out = np.empty((B, N2), np.float32)
    for i in range(NCORES):
        arr = res.results[i]["out_d"]                              # [128, 1152]
        xc = arr.reshape(128, NT, 128).transpose(2, 1, 0).reshape(BL, N2P)
        out[i * BL:(i + 1) * BL] = xc[:, :N2]
    return out
